# revision 21
# baseline (speedup 1.0000x reference)
"""Single-query cross-attention (B=16, S=4096, D=1024, H=16) on 8 TRN2 cores.

Math fold: for query length 1,
    scores[b,h,s] = (Wk_h^T q_h[b]) . enc[b,s,:] / sqrt(hd)   (q-tilde trick)
    ctx[b,h,:]    = Wv_h @ (sum_s w[b,h,s] enc[b,s,:])        (Wv fold)
so the big K/V projections (275 GFLOP) are never materialized; the kernel
streams encoder_outputs once per layout.  Batch is sharded 2-per-core; no
collectives.  Host-side prep is layout/dtype only (no math): bf16/fp8
casts, weight transposes, a transposed copy of enc for the scores
contraction, and an s-permutation baked into both enc layouts (softmax and
the weighted sum are s-permutation invariant).

Key structure:
  - natural-layout enc (c-tilde side) is fp8e4m3, packed 4 s-rows per
    partition (4 KB DMA descriptors); the ctx matmul consumes it
    mixed-dtype against bf16 attention weights.
  - scores are computed TRANSPOSED: scT[s,h] = sum_d encT[d,s] qtT[d,h]
    with the encT block as the stationary operand.  The [128s, H] result
    is exactly the layout the ctx matmul needs, so no per-tile PE
    transposes of w; exp runs straight PSUM->SBUF; the softmax
    denominator comes from a ones-vector matmul accumulating in PSUM.
  - prologue: dhT + per-jb weight slices DMA first, q/q-tildeT matmuls
    consume each 128-row slice as it lands; q-tildeT is produced by a
    block-diagonal-masked contraction (the zero blocks do the per-head
    masking).
"""

import sys
import numpy as np

for _p in ("/opt/trn_rl_repo",):
    if _p not in sys.path:
        sys.path.insert(0, _p)

import ml_dtypes
import concourse.bass as bass
import concourse.bacc as bacc
import concourse.tile as tile
from concourse import mybir
from concourse.masks import make_identity
from concourse.bass_utils import run_bass_kernel_spmd

B, S, D, H = 16, 4096, 1024, 16
HD = D // H                      # 64
NCORES = 8
BPC = B // NCORES                # 2 batches per core
NJ = D // 128                    # 8 d-blocks
GRP = 4                          # s-tiles (128) per group = 512 positions
SQ = 512                         # encT slab width (= one group)
PACK = 8                         # s-rows packed per partition in fp8 pair tiles

F32 = mybir.dt.float32
BF16 = mybir.dt.bfloat16
FP8 = mybir.dt.float8e3   # e3m4: 4 mantissa bits, range +-15.5 (randn fits)
EDT = FP8                 # natural enc (c-tilde rhs)
SDT = FP8                 # transposed enc (scores lhsT); was bf16
SC_SCALE = 2.5            # host premultiply before e3m4 cast (pushes values out
V_SCALE = 2.0             # of the subnormal range); compensated in qt / linv
EXP_BIAS = -2.0           # exp(s-2); cancels in 1/l
DEBUG_DUMP = False        # dump cT_m + linv_bc to a dbg DRAM tensor (sim)


def build_nc(s=S):
    nc = bacc.Bacc(None, target_bir_lowering=False, debug=False)

    # enc: fp8, pair-packed: [b, pair, partition, 8 KB row], row = 8 s-rows
    # (s = pair*1024 + 8*p + cls).  encT: bf16, slab-major: [b, slab,
    # partition, jb, t] so each partition row is 8 KB contiguous DRAM.
    dhT_ext = nc.declare_dram_parameter("dhT", [D, BPC], BF16, isOutput=False)
    enc_ext = nc.declare_dram_parameter(
        "enc", [BPC, s // 1024, 128, PACK * D], EDT, isOutput=False
    )
    encT_ext = nc.declare_dram_parameter(
        "encT", [BPC, s // SQ, 128, NJ, SQ], SDT, isOutput=False
    )
    wqT_ext = nc.declare_dram_parameter("wqT", [D, D], BF16, isOutput=False)
    wk_ext = nc.declare_dram_parameter("wk", [D, D], BF16, isOutput=False)
    wvT_ext = nc.declare_dram_parameter("wvT", [D, D], BF16, isOutput=False)
    out_ext = nc.declare_dram_parameter("out", [BPC, D], F32, isOutput=True)
    dbg_ext = None
    if DEBUG_DUMP:
        dbg_ext = nc.declare_dram_parameter(
            "dbg", [128, NJ * H * BPC + 2 * H], F32, isOutput=True
        )

    with tile.TileContext(nc) as tc:
        _build(nc, tc, s, dhT_ext, enc_ext, encT_ext, wqT_ext, wk_ext, wvT_ext,
               out_ext, dbg_ext)
    nc.compile()
    return nc


def _build(nc, tc, s, dhT_ext, enc_ext, encT_ext, wqT_ext, wk_ext, wvT_ext,
           out_ext, dbg_ext=None):
    NT = s // 128                # s-tiles per batch
    NG = NT // GRP               # groups (of 512 positions) per batch
    from contextlib import ExitStack

    ctx = ExitStack()
    with ctx:
        singles = ctx.enter_context(tc.tile_pool(name="singles", bufs=1))
        wjb = ctx.enter_context(tc.tile_pool(name="wjb", bufs=1))
        slabp = ctx.enter_context(tc.tile_pool(name="slabp", bufs=12))
        epool = ctx.enter_context(tc.tile_pool(name="epool", bufs=8))
        sc = ctx.enter_context(tc.tile_pool(name="sc", bufs=2))
        wts = ctx.enter_context(tc.tile_pool(name="wts", bufs=12))
        pp_bf = ctx.enter_context(tc.tile_pool(name="pp_bf", bufs=2, space="PSUM"))
        pp_f32 = ctx.enter_context(tc.tile_pool(name="pp_f32", bufs=1, space="PSUM"))
        # one bank each for the packed scores slots and the packed c-tilde
        # accumulators (tags get separate rings, so bufs=1 is per-tag)
        pp_str = ctx.enter_context(tc.tile_pool(name="pp_str", bufs=1, space="PSUM"))

        # ---- inputs: dhT first (tiny, unblocks q matmuls), then per-jb
        # slices of WqT and Wk so the prologue pipelines with the DMA.
        dhT_sb = singles.tile([128, NJ, BPC], BF16, tag="dhT")
        nc.sync.dma_start(
            out=dhT_sb, in_=dhT_ext[:, :].rearrange("(jb p) b -> p jb b", p=128)
        )
        wq_jb = []
        wk_jb = []
        for jb in range(NJ):
            t = wjb.tile([128, D], BF16, tag=f"wq{jb}")
            nc.sync.dma_start(out=t, in_=wqT_ext[jb * 128:(jb + 1) * 128, :])
            wq_jb.append(t)
        for jb in range(NJ):
            t = wjb.tile([128, D], BF16, tag=f"wk{jb}")
            nc.sync.dma_start(out=t, in_=wk_ext[jb * 128:(jb + 1) * 128, :])
            wk_jb.append(t)

        # ---- enc streaming: encT slabs (one group wide) and pair-packed fp8
        # natural tiles, both prefetched ahead of use.  All host-packed so
        # every DMA descriptor is a >=4 KB contiguous DRAM read.  Prefetch is
        # deep so the DMA engines stay saturated while the PE waits for qt.
        NPAIR = NG // 2
        slabs = [[None] * NG for _ in range(BPC)]
        e8s = [[None] * NPAIR for _ in range(BPC)]

        def load_slab(b, g):
            if slabs[b][g] is None:
                t = slabp.tile([128, NJ, SQ], SDT, tag="slab")
                nc.sync.dma_start(out=t, in_=encT_ext[b, g])
                slabs[b][g] = t
            return slabs[b][g]

        def load_e4(b, g):
            pr = g // 2
            if e8s[b][pr] is None:
                t = epool.tile([128, PACK * D], EDT, tag="e4")
                nc.sync.dma_start(out=t, in_=enc_ext[b, pr])
                e8s[b][pr] = t
            return e8s[b][pr]

        for g0 in range(min(4, NG)):
            for b in range(BPC):
                load_slab(b, g0)
            for b in range(BPC):
                load_e4(b, g0)

        # ---- constants
        ident = singles.tile([128, 128], BF16)
        make_identity(nc, ident)

        # ---- q[b, i] = sum_d dh[b, d] Wq[i, d]; consume wq slices as they land
        q_ps = pp_f32.tile([BPC, D], F32, tag="pf32")
        for jb in range(NJ):
            for chunk in range(2):
                cs = slice(chunk * 512, (chunk + 1) * 512)
                nc.tensor.matmul(
                    q_ps[:, cs],
                    dhT_sb[:, jb, :],
                    wq_jb[jb][:, cs],
                    start=(jb == 0),
                    stop=(jb == NJ - 1),
                    skip_group_check=True,
                )
        q_sb = singles.tile([BPC, D], BF16, tag="q")
        nc.vector.tensor_copy(out=q_sb, in_=q_ps)

        # qhT: block-diagonal [i, r], r = h*2 + b; qhT[i, r] = q[b, i] iff head(i)==h.
        qhT_sb = singles.tile([128, NJ, 2 * H], BF16, tag="qhT")
        nc.vector.memset(qhT_sb, 0.0)
        for jb in range(NJ):
            ps = pp_bf.tile([128, 128], BF16, tag="ppsum_big")
            nc.tensor.transpose(
                ps[:, 0:BPC], q_sb[:, jb * 128:(jb + 1) * 128], ident[0:BPC, 0:BPC]
            )
            nc.vector.tensor_copy(
                out=qhT_sb[0:64, jb, 4 * jb:4 * jb + 2], in_=ps[0:64, 0:BPC]
            )
            nc.vector.tensor_copy(
                out=qhT_sb[64:128, jb, 4 * jb + 2:4 * jb + 4], in_=ps[64:128, 0:BPC]
            )

        # q-tildeT directly: qtT[d', r] = sum_i Wk[i, d'] qhT[i, r]; the
        # block-diagonal zeros in qhT mask the contraction to head(i)==h(r).
        qtT_all = singles.tile([128, NJ, 2 * H], BF16, tag="qtT_all")
        for db in range(NJ):
            qt_ps = pp_bf.tile([128, 64], F32, tag="ppsum_big")
            for jb_i in range(NJ):
                nc.tensor.matmul(
                    qt_ps[:, 0:2 * H],
                    wk_jb[jb_i][:, db * 128:(db + 1) * 128],
                    qhT_sb[:, jb_i, :],
                    start=(jb_i == 0),
                    stop=(jb_i == NJ - 1),
                )
            nc.vector.tensor_scalar_mul(
                qtT_all[:, db, :], qt_ps[:, 0:2 * H], 1.0 / (np.sqrt(HD) * SC_SCALE)
            )
        qtT_b = []
        qtT_v = qtT_all.rearrange("p j (h b) -> p j h b", b=BPC)
        for b in range(BPC):
            t = singles.tile([128, NJ, H], BF16, tag=f"qtT{b}")
            nc.vector.tensor_copy(out=t, in_=qtT_v[:, :, :, b])
            qtT_b.append(t)

        # ---- main streaming loop (single pass, unnormalized-exp softmax)
        ebias = singles.tile([128, 1], F32, tag="ebias")
        nc.vector.memset(ebias, EXP_BIAS)
        # one V_SCALE-valued column: the l-fold matmul yields V_SCALE*l, so
        # 1/linv also undoes the host-side e3m4 quantization scale of enc.
        onescol = singles.tile([128, 1], BF16, tag="onescol")
        nc.vector.memset(onescol, V_SCALE)
        F16 = mybir.dt.float16
        ones_row = singles.tile([1, 128], F16, tag="ones_row")
        nc.vector.memset(ones_row, 1.0)

        # scores PSUM: 8 rotating [128, H] slots packed into one bank.
        NSLOT = 8
        scT_all = pp_str.tile([128, NSLOT * 2 * H], F32, tag="scT_pack")
        # c-tilde PSUM: one [128, H] accumulator per (b, jb), all in one bank.
        # ctx matmuls run enc-natural-stationary (fp8 LDW is cheap via FWL),
        # so c-tilde lands already TRANSPOSED: cT[d, h] per (b, jb).
        c_all = pp_str.tile([128, BPC * NJ * H], F32, tag="c_pack")

        l_acc = []
        for b in range(BPC):
            la = sc.tile([128, H], F32, tag=f"lacc{b}")
            nc.vector.memset(la, 0.0)
            l_acc.append(la)
        wvT_sb = None
        # ctx matmuls are emitted CTX_LAG tiles late: the PE queue is
        # in-order, so emitting ctx(t) right after scores(t) stalls the PE
        # on the exp(t) scalar round-trip every tile.  With the lag, exp
        # latency hides behind a full tile of PE scores work.
        CTX_LAG = 4
        pending = []

        # PSUM start=True marks the whole 2 KB zero-region (bank) pending-zero;
        # the first matmul TOUCH of a pending byte overwrites instead of
        # accumulating.  With 16 interleaved (b, jb) accumulators packed into
        # one bank, only the chronologically-first ctx matmul may carry
        # start=True -- each slot's own first write then lands on
        # still-pending bytes and initializes itself.
        ctx_started = [False]

        def emit_ctx(item):
            b2, wt2, e42, ecls2, first2, last2 = item
            for jb in range(NJ):
                nc.tensor.matmul(
                    c_all[:, (b2 * NJ + jb) * H:(b2 * NJ + jb + 1) * H],
                    e42[:, ecls2 * D + jb * 128:ecls2 * D + (jb + 1) * 128],
                    wt2,
                    start=not ctx_started[0],
                    stop=last2,
                    skip_group_check=True,
                )
                ctx_started[0] = True

        for g in range(NG):
            for b in range(BPC):
                if g + 4 < NG:
                    load_slab(b, g + 4)
                    load_e4(b, g + 4)
                if g == min(2, NG - 1) and b == 0 and wvT_sb is None:
                    wvT_sb = singles.tile([128, NJ, D], BF16, tag="wvT")
                    nc.sync.dma_start(
                        out=wvT_sb,
                        in_=wvT_ext[:, :].rearrange("(jb p) d -> p jb d", p=128),
                    )
                slab = load_slab(b, g)
                e4 = load_e4(b, g)
                for cls in range(GRP):
                    t = g * GRP + cls
                    slot = (t * BPC + b) % NSLOT
                    scT_ps = scT_all[:, slot * 2 * H:slot * 2 * H + H]
                    # scT[s, h] = sum_d encT[d, s] qtT[d, h] for this 128-block
                    for jb in range(NJ):
                        nc.tensor.matmul(
                            scT_ps,
                            slab[:, jb, cls * 128:(cls + 1) * 128],
                            qtT_b[b][:, jb, :],
                            start=(jb == 0),
                            stop=(jb == NJ - 1),
                            skip_group_check=True,
                        )
                    # unnormalized weights straight from PSUM, already [s, h]
                    wt_t = wts.tile([128, H], BF16, tag="wt")
                    nc.scalar.activation(
                        out=wt_t,
                        in_=scT_ps,
                        func=mybir.ActivationFunctionType.Exp,
                        bias=ebias,
                    )
                    first = t == 0
                    last = t == NT - 1
                    # softmax denominator partials on the (idle) DVE
                    la2 = l_acc[b]
                    nc.vector.tensor_add(la2[:, 0:H], la2[:, 0:H], wt_t)
                    ecls = 4 * (g % 2) + cls
                    pending.append((b, wt_t, e4, ecls, first, last))
                    if len(pending) > CTX_LAG:
                        emit_ctx(pending.pop(0))
        while pending:
            emit_ctx(pending.pop(0))

        # ---- epilogue: linv = 1/(V_SCALE*l) broadcast down partitions, then
        # normalize+cast cT straight out of PSUM, then the Wv fold.
        linv_bc = []
        for b in range(BPC):
            lbf = sc.tile([128, H], BF16, tag=f"lbf{b}")
            nc.vector.tensor_copy(out=lbf, in_=l_acc[b])
            lrow_ps = pp_bf.tile([1, 64], F32, tag="ppsum_small")
            nc.tensor.matmul(lrow_ps[:, 0:H], onescol, lbf)
            linv_f32 = sc.tile([1, H], F32, tag=f"linvf{b}")
            nc.vector.reciprocal(linv_f32, lrow_ps[:, 0:H])
            linv_row = sc.tile([1, H], F16, tag=f"linvrow{b}")
            nc.vector.tensor_copy(out=linv_row, in_=linv_f32)
            bc_ps = pp_bf.tile([128, 64], F32, tag="ppsum_small")
            nc.tensor.matmul(bc_ps[:, 0:H], ones_row, linv_row)
            bc = sc.tile([128, H], F32, tag=f"linvbc{b}")
            nc.vector.tensor_copy(out=bc, in_=bc_ps[:, 0:H])
            linv_bc.append(bc)

        # cT_m[d, jb, h, b]: normalized bf16 c-tilde, fold-ready layout.
        cT_m = singles.tile([128, NJ, H, BPC], BF16, tag="cTm")
        for jb in range(NJ):
            for b in range(BPC):
                nc.vector.tensor_mul(
                    cT_m[:, jb, :, b],
                    c_all[:, (b * NJ + jb) * H:(b * NJ + jb + 1) * H],
                    linv_bc[b],
                )
        # Wv fold: out[b, i] = sum_d cT[d, h(i), b] * WvT[d, i], per head.
        fold_ps = pp_f32.tile([BPC, H * HD], F32, tag="pf32")
        for h in range(H):
            hs = slice(h * HD, (h + 1) * HD)
            for jb in range(NJ):
                nc.tensor.matmul(
                    fold_ps[:, hs],
                    cT_m[:, jb, h, :],
                    wvT_sb[:, jb, hs],
                    start=(jb == 0),
                    stop=(jb == NJ - 1),
                    skip_group_check=True,
                )
        if dbg_ext is not None:
            dbg_sb = singles.tile([128, NJ * H * BPC + 2 * H], F32, tag="dbg_sb")
            nc.vector.tensor_copy(
                out=dbg_sb[:, 0:NJ * H * BPC],
                in_=cT_m.rearrange("p j h b -> p (j h b)"),
            )
            for b in range(BPC):
                nc.vector.tensor_copy(
                    out=dbg_sb[:, NJ * H * BPC + b * H:NJ * H * BPC + (b + 1) * H],
                    in_=linv_bc[b],
                )
            nc.sync.dma_start(out=dbg_ext[:, :], in_=dbg_sb)
        ob = singles.tile([BPC, D], F32, tag="out_sb")
        nc.vector.tensor_copy(out=ob[:, 0:512], in_=fold_ps[:, 0:512])
        nc.scalar.activation(
            out=ob[:, 512:D], in_=fold_ps[:, 512:D],
            func=mybir.ActivationFunctionType.Identity,
        )
        nc.sync.dma_start(out=out_ext[:, :], in_=ob)


_NC_CACHE = None


def _get_nc():
    global _NC_CACHE
    if _NC_CACHE is None:
        _NC_CACHE = build_nc()
    return _NC_CACHE


def _sperm(s):
    """Within each 1024-block: order positions by class (s mod 8), so scores
    tiles match the pair-packed fp8 partition order s = base + 8*p + cls."""
    perm = np.empty(s, dtype=np.int64)
    i = 0
    for base in range(0, s, 1024):
        n = min(1024, s - base)
        for cls in range(PACK):
            for p in range(n // PACK):
                perm[i] = base + p * PACK + cls
                i += 1
    return perm


def _shard(inputs):
    """Host-side prep: shard batch, cast dtypes, pre-transpose layouts."""
    bf = ml_dtypes.bfloat16
    e3 = ml_dtypes.float8_e3m4
    dh = np.asarray(inputs["decoder_hidden"], dtype=np.float32)
    enc = np.asarray(inputs["encoder_outputs"], dtype=np.float32)
    wqT = np.ascontiguousarray(np.asarray(inputs["Wq"], dtype=np.float32).T).astype(bf)
    wk = np.ascontiguousarray(np.asarray(inputs["Wk"], dtype=np.float32)).astype(bf)
    wvT = np.ascontiguousarray(np.asarray(inputs["Wv"], dtype=np.float32).T).astype(bf)
    enc_c = np.clip(enc * V_SCALE, -15.5, 15.5).astype(e3)
    enc_s = np.clip(enc * SC_SCALE, -15.5, 15.5).astype(e3)
    perm = _sperm(enc.shape[1])
    in_maps = []
    Sfull = enc.shape[1]
    for c in range(NCORES):
        sl = slice(c * BPC, (c + 1) * BPC)
        dhT = np.ascontiguousarray(dh[sl].T).astype(bf)
        # fp8 pair-packed: [b, pair, partition, (cls d)] -- 8 KB rows
        eb = np.ascontiguousarray(
            enc_c[sl].reshape(BPC, Sfull // 1024, 128, PACK * D)
        )
        # fp8 slab-major: [b, slab, partition, jb, t] -- 4 KB rows
        ebT = enc_s[sl].transpose(0, 2, 1)[:, :, perm]
        ebT = np.ascontiguousarray(
            ebT.reshape(BPC, NJ, 128, Sfull // SQ, SQ).transpose(0, 3, 2, 1, 4)
        )
        in_maps.append(
            {
                "dhT": dhT,
                "enc": eb,
                "encT": ebT,
                "wqT": wqT,
                "wk": wk,
                "wvT": wvT,
            }
        )
    return in_maps


def _run(inputs, trace=False, **kw):
    global _NC_CACHE
    in_maps = _shard(inputs)
    last_err = None
    for attempt in range(3):
        try:
            nc = _get_nc()
            res = run_bass_kernel_spmd(
                nc, in_maps, core_ids=list(range(NCORES)), trace=trace, **kw
            )
            out = np.concatenate([np.asarray(r["out"]) for r in res.results], axis=0)
            return out.astype(np.float32), res
        except Exception as e:  # transient NRT_EXEC_UNIT_UNRECOVERABLE etc.
            last_err = e
            _NC_CACHE = None  # rebuild the graph fresh on retry
            import time
            time.sleep(2.0)
    raise last_err


def kernel(**inputs):
    out, _ = _run(inputs, trace=False)
    return out

